# revision 41
# baseline (speedup 1.0000x reference)
"""DiT block kernel for 8x Trainium2 NeuronCores (Bass/Tile).

Sharding: row-parallel over the flattened (B,T)=4096 rows; 512 rows/core.
Cores 0-3 handle batch 0, cores 4-7 batch 1. MQA K/V is computed per-shard
and AllGather'd within each 4-core batch group. Weights are replicated and
cast to bf16; LN/residual math stays fp32.

v3 notes (on top of v2's row-tiled MM1/outproj, 2-bank exp, prefetch):
  - attention is software-pipelined: MM1 of quad k+1 issues before PV of
    quad k so the scalar-engine exp stream never starves; the per-pair
    softmax tail (denominator copies -> one batched reciprocal -> bcr
    broadcast matmul -> output muls) is deferred into the next pair so
    the slow DVE reciprocal never stalls the PE FIFO.
  - attn-LN's gamma/beta are folded into wq/wkv host-side (bias lands
    via the per-partition bias operand of the qT/kvT PSUM-copy), so the
    second LN is just normalize (saves 2 DVE passes/row-block + 8KB).
  - kvT bounce rides the gpsimd queue right before the AllGather, so the
    collective no longer waits behind megabytes of weight DMA.
  - DMA queues balanced so x/wmod/wq all land before their consumers.
"""

import os
import sys

sys.path.insert(0, "/opt/trn_rl_repo")

import numpy as np
import ml_dtypes

BF16 = ml_dtypes.bfloat16

B, T, F, H, D, M, C = 2, 2048, 1024, 16, 64, 4, 512
NCORES = 8
R = (B * T) // NCORES  # 512 rows per core
RB = R // 128  # 4 row blocks
FT = F // 128  # 8 feature tiles
MT = (H * D) // 128  # 8 head-pair tiles
MFT = (M * F) // 128  # 32 mlp hidden tiles
KT = T // 128  # 16 key tiles
EPS = 1e-5

_CACHE = {}


def _build_nc():
    import concourse.bass as bass
    import concourse.tile as tile
    from concourse import bacc, mybir
    from concourse.masks import make_identity
    from contextlib import ExitStack

    f32 = mybir.dt.float32
    f16 = mybir.dt.float16
    bf16 = mybir.dt.bfloat16
    AF = mybir.ActivationFunctionType
    OP = mybir.AluOpType

    STOP = int(os.environ.get("STOP_AFTER", "99"))

    nc = bacc.Bacc(
        "TRN2",
        target_bir_lowering=False,
        debug=False,
        enable_asserts=False,
        num_devices=NCORES,
    )

    def dram(name, shape, dt, **kw):
        return nc.dram_tensor(name, shape, dt, **kw).ap()

    x_d = dram("x", [T, F], f32, kind="ExternalInput")
    cond_d = dram("cond", [C], bf16, kind="ExternalInput")
    wmod_d = dram("wmod", [C, 4 * F], bf16, kind="ExternalInput")
    modb_d = dram("modb", [4 * F], bf16, kind="ExternalInput")
    lnv16_d = dram("lnvec16", [6, F], bf16, kind="ExternalInput")
    wq_d = dram("wq", [MT, 128, FT * 128], bf16, kind="ExternalInput")
    qb_d = dram("qbias", [H * D], f32, kind="ExternalInput")
    wkv_d = dram("wkv", [F, 2 * D], bf16, kind="ExternalInput")
    kvb_d = dram("kvbias", [2 * D, 1], f32, kind="ExternalInput")
    # wo pre-paired: [pair, 128(d of even head | d of odd head), F]
    wo_d = dram("wo", [MT, 128, F], bf16, kind="ExternalInput")
    wob_d = dram("wo_bias", [1, F], bf16, kind="ExternalInput")
    w1_d = dram("w1", [MFT, 128, FT * 128], bf16, kind="ExternalInput")
    b1_d = dram("b1", [M * F], f32, kind="ExternalInput")
    w2_d = dram("w2", [M * F, F], bf16, kind="ExternalInput")
    b2_d = dram("b2", [1, F], bf16, kind="ExternalInput")
    y_d = dram("y", [R, F], f32, kind="ExternalOutput")

    groups = [[0, 1, 2, 3], [4, 5, 6, 7]]

    def bcast_row(ap_row):
        # [1, n] DRAM AP -> partition-broadcast [128, n]
        return bass.AP(
            tensor=ap_row.tensor,
            offset=ap_row.offset,
            ap=[[0, 128]] + list(ap_row.ap[-1:]),
        )

    with tile.TileContext(nc) as tc, ExitStack() as ctx:
        # left stack: consts, work, hTp, w1p, hera, aera, wop, [attnp]
        # right stack: wqp, xp, [modtmp], then x1p, g1p
        consts = ctx.enter_context(tc.tile_pool(name="consts", bufs=1))
        work = ctx.enter_context(tc.tile_pool(name="work", bufs=2))
        cm_hTp = tc.tile_pool(name="hTp", bufs=1)
        hTp = cm_hTp.__enter__()
        cm_w1p = tc.tile_pool(name="w1p", bufs=1)
        w1p = cm_w1p.__enter__()
        cm_hera = tc.tile_pool(name="hera", bufs=1)
        hera = cm_hera.__enter__()
        cm_wqp = tc.tile_pool(name="wqp", bufs=1, side="right")
        wqp = cm_wqp.__enter__()
        cm_xp = tc.tile_pool(name="xp", bufs=1, side="right")
        xp = cm_xp.__enter__()
        cm_modtmp = tc.tile_pool(name="modtmp", bufs=1, side="right")
        modtmp = cm_modtmp.__enter__()
        dramp = ctx.enter_context(tc.tile_pool(name="dramp", bufs=1, space="DRAM"))

        # ---------------- constants ----------------
        ident = consts.tile([128, 128], bf16, name="ident")
        make_identity(nc, ident)
        ones16 = consts.tile([1, 128], f16, name="ones16")
        nc.vector.memset(ones16, 1.0)
        onescol = consts.tile([1, 128], bf16, name="onescol")
        nc.vector.memset(onescol, 1.0)
        epst = consts.tile([128, 1], f32, name="epst")
        nc.vector.memset(epst, EPS)
        # ones rows at partitions 64 (even-head denom) and 32 (odd-head denom)
        ones2 = consts.tile([128, 64], f16, name="ones2")
        nc.vector.memset(ones2[64:65, :], 1.0)
        nc.vector.memset(ones2[32:33, :], 1.0)

        # cond + wmod grp0 first on both queues (the Wa/Ba modulation
        # gate phase 1's DVE chain), x0 next, then the rest.
        cond_sb = consts.tile([128, 4], bf16, name="cond_sb")
        nc.sync.dma_start(out=cond_sb, in_=cond_d.rearrange("(a p) -> p a", p=128))
        # wmod chunks: grp0 split across sync (ch0/ch2, interleaved with x)
        # and scalar (ch1/ch3); grp1 on scalar. x row blocks on sync.
        def wm_dma(eng, grp, ch):
            wm = modtmp.tile([128, 2048], bf16, tag="wm", bufs=4, name=f"wm{grp}{ch}")
            eng.dma_start(
                out=wm,
                in_=wmod_d[ch * 128 : (ch + 1) * 128, grp * 2048 : (grp + 1) * 2048],
            )
            return wm

        xs = []

        def x_dma(rb):
            x_rb = xp.tile([128, F], f32, name=f"x{rb}")
            nc.sync.dma_start(out=x_rb, in_=x_d[rb * 128 : (rb + 1) * 128, :])
            xs.append(x_rb)

        wm0 = {}
        wm0[1] = wm_dma(nc.scalar, 0, 1)
        wm0[3] = wm_dma(nc.scalar, 0, 3)
        wm0[0] = wm_dma(nc.sync, 0, 0)
        x_dma(0)
        wm0[2] = wm_dma(nc.sync, 0, 2)
        for rb in (1, 2, 3):
            x_dma(rb)
        wm1 = {ch: wm_dma(nc.scalar, 1, ch) for ch in range(4)}
        # wq split across both queues; lands ~45us, q-proj starts ~50us
        wq_sb = wqp.tile([128, MT, FT * 128], bf16, name="wq_sb")
        for mt in range(4):
            nc.sync.dma_start(out=wq_sb[:, mt, :], in_=wq_d[mt])
        for mt in range(4, MT):
            nc.scalar.dma_start(out=wq_sb[:, mt, :], in_=wq_d[mt])

        b1_sb = consts.tile([128, MFT], f32, name="b1_sb")
        nc.sync.dma_start(out=b1_sb, in_=b1_d.rearrange("(mt p) -> p mt", p=128))
        wob_sb = consts.tile([1, F], bf16, name="wob_sb")
        nc.sync.dma_start(out=wob_sb, in_=wob_d)
        b2_sb = consts.tile([1, F], bf16, name="b2_sb")
        nc.sync.dma_start(out=b2_sb, in_=b2_d)
        qb_sb = consts.tile([128, MT], f32, name="qb_sb")
        nc.sync.dma_start(out=qb_sb, in_=qb_d.rearrange("(mt p) -> p mt", p=128))
        kvb_sb = consts.tile([128, 1], f32, name="kvb_sb")
        nc.sync.dma_start(out=kvb_sb, in_=kvb_d)
        lnr = {}
        for r in (0, 1, 4, 5):  # amod_nw/nb, fmod_nw/nb rows at partition 0
            lnr[r] = modtmp.tile([1, F], bf16, name=f"lnr{r}")
            nc.sync.dma_start(out=lnr[r], in_=lnv16_d[r : r + 1, :])
        modb_sb = modtmp.tile([1, 4 * F], bf16, name="modb_sb")
        nc.sync.dma_start(out=modb_sb, in_=modb_d.rearrange("(a f) -> a f", a=1))
        wkv_sb = consts.tile([128, FT, 2 * D], bf16, name="wkv_sb")
        nc.sync.dma_start(
            out=wkv_sb, in_=wkv_d.rearrange("(kt p) n -> p kt n", p=128)
        )


        # ---------------- phase 0: modulation vectors ----------------
        modv = modtmp.tile([1, 4 * F], f16, name="modv")
        tmpv = modtmp.tile([1, F], f16, name="tmpv")
        bc = {}

        def filler(pool, n):
            # dummy matmuls: keep the PE HAM activity window busy
            for _ in range(n):
                wps = pool.tile([128, 512], f32, tag="pmod", bufs=2, name="warm")
                nc.tensor.matmul(wps[:, 0:128], ident, ident, start=True, stop=True)

        def mod_matmuls(ps_pool, grp, wms):
            for jp in range(2):
                pms = [
                    ps_pool.tile([128, 512], f32, tag="pmod", bufs=2, name=f"pm{j}")
                    for j in range(2)
                ]
                for ch in range(4):
                    for j in range(2):
                        nc.tensor.matmul(
                            pms[j][0:1, :],
                            cond_sb[:, ch : ch + 1],
                            wms[ch][:, (2 * jp + j) * 512 : (2 * jp + j + 1) * 512],
                            start=(ch == 0),
                            stop=(ch == 3),
                        )
                with nc.allow_low_precision(reason="f16 modulation vector"):
                    for j in range(2):
                        nb = grp * 4 + 2 * jp + j
                        nc.vector.tensor_add(
                            out=modv[:, nb * 512 : (nb + 1) * 512],
                            in0=pms[j][0:1, :],
                            in1=modb_sb[:, nb * 512 : (nb + 1) * 512],
                        )

        def finalize_mod(ps_pool, g_off, b_off, nw_row, nb_row, w_name, b_name):
            g_sl = modv[:, g_off : g_off + F]
            b_sl = modv[:, b_off : b_off + F]
            with nc.allow_low_precision(reason="f16 modulation vector"):
                nc.scalar.add(out=g_sl, in_=g_sl, add=1.0)
            with nc.allow_low_precision(reason="f16 staging for PE broadcast"):
                nc.vector.tensor_mul(out=tmpv, in0=g_sl, in1=lnr[nb_row])
                nc.vector.tensor_add(out=b_sl, in0=tmpv, in1=b_sl)
                nc.vector.tensor_mul(out=g_sl, in0=g_sl, in1=lnr[nw_row])
            for off, nm in ((g_off, w_name), (b_off, b_name)):
                # Wa/Ba feed f32 DVE/gpsimd chains: mixed-dtype tensor ops
                # run ~3x slower, so keep those two in f32
                bdt = f32 if nm in ("Wa_bc", "Ba_bc") else bf16
                bt = consts.tile([128, F], bdt, name=nm)
                for hf in range(2):
                    pb = ps_pool.tile([128, 512], f32, tag="pmod", bufs=2, name="pbc")
                    nc.tensor.matmul(
                        pb,
                        ones16,
                        modv[:, off + hf * 512 : off + (hf + 1) * 512],
                        start=True,
                        stop=True,
                    )
                    nc.scalar.activation(
                        bt[:, hf * 512 : (hf + 1) * 512], pb, AF.Copy
                    )
                bc[nm] = bt

        cm_ps1t = tc.tile_pool(name="ps1t", bufs=1, space="PSUM")
        ps1t = cm_ps1t.__enter__()
        cm_ps1a = tc.tile_pool(name="ps1a", bufs=1, space="PSUM")
        ps1a = cm_ps1a.__enter__()

        filler(ps1a, 40)
        mod_matmuls(ps1a, 0, wm0)
        finalize_mod(ps1a, 0, F, 0, 1, "Wa_bc", "Ba_bc")

        # ---------------- helpers ----------------
        def layer_norm(src, w_bc, b_bc, out_tile, badd_engine):
            """out = LN(src) * w_bc + b_bc ; src [128,F] f32.
            If w_bc is None: out = plain LN(src) (bf16 ok)."""
            stats = work.tile([128, 2, 6], f32, tag="stats", name="stats")
            for sg in range(2):
                nc.vector.bn_stats(
                    out=stats[:, sg, :], in_=src[:, sg * 512 : (sg + 1) * 512]
                )
            mv = work.tile([128, 2], f32, tag="mv", name="mv")
            nc.vector.bn_aggr(out=mv, in_=stats)
            rstd = work.tile([128, 1], f32, tag="rstd", name="rstd")
            nc.scalar.activation(
                out=rstd, in_=mv[:, 1:2], func=AF.Sqrt, bias=epst, scale=1.0
            )
            nc.vector.reciprocal(out=rstd, in_=rstd)
            tgt = out_tile if w_bc is None else work.tile(
                [128, F], f32, tag="xn", bufs=1, name="xn"
            )
            with nc.allow_low_precision(reason="bf16 normalized activations"):
                nc.vector.tensor_scalar(
                    out=tgt,
                    in0=src,
                    scalar1=mv[:, 0:1],
                    scalar2=rstd,
                    op0=OP.subtract,
                    op1=OP.mult,
                )
            if w_bc is None:
                return
            nc.vector.tensor_mul(out=tgt, in0=tgt, in1=w_bc)
            badd_engine.tensor_add(out=out_tile, in0=tgt, in1=b_bc)

        def transpose_to(ps_pool, bufs, hsrc_bf, hT_tiles, rb):
            """hsrc_bf [128,F] bf16 -> hT_tiles[ft][:, rb*128:+128]."""
            for ft in range(FT):
                pt = ps_pool.tile([128, 128], bf16, tag="ptt", bufs=bufs, name="ptt")
                nc.tensor.transpose(
                    pt, hsrc_bf[:, ft * 128 : (ft + 1) * 128], ident
                )
                nc.scalar.activation(
                    out=hT_tiles[ft][:, rb * 128 : (rb + 1) * 128],
                    in_=pt,
                    func=AF.Copy,
                )

        hT = [
            hTp.tile([128, R], bf16, tag=f"hT{ft}", name=f"hT{ft}")
            for ft in range(FT)
        ]

        # ---------------- phase 1: adaLN-1 + attn-LN + transpose ----------------
        h_res = [hera.tile([128, F], f32, name=f"h{rb}") for rb in range(RB)]
        # pass 1: plain LN of x into h_res -- no dependence on Wa/Ba, so the
        # DVE starts as soon as x lands (the wmod chunks are still arriving)
        for rb in range(RB):
            layer_norm(xs[rb], None, None, h_res[rb], None)
        # pass 2: modulation in place (mul on DVE, add on gpsimd), then the
        # attention-internal LN + transpose
        for rb in range(RB):
            nc.vector.tensor_mul(out=h_res[rb], in0=h_res[rb], in1=bc["Wa_bc"])
            nc.gpsimd.tensor_add(out=h_res[rb], in0=h_res[rb], in1=bc["Ba_bc"])
        for rb in range(RB):
            hn_bf = work.tile([128, F], bf16, tag="hnbf", bufs=2, name="hn_bf")
            layer_norm(h_res[rb], None, None, hn_bf, None)
            transpose_to(ps1t, 2, hn_bf, hT, rb)
            if rb == 1:
                # fmod modulation slotted here: wm grp1 has landed, and the
                # small DVE/PE work hides between the LN chains
                mod_matmuls(ps1a, 1, wm1)
                finalize_mod(ps1a, 2 * F, 3 * F, 4, 5, "Wf_bc", "Bf_bc")
                cm_modtmp.__exit__(None, None, None)

        cm_xp.__exit__(None, None, None)
        cm_ps1a.__exit__(None, None, None)

        # ---------------- phase 2: q projection + local K/V ----------------
        # no collective: x is the per-core ROTATED full batch (own shard
        # first), so every core computes K/V for all 2048 keys itself.
        # Key order differs per core, but softmax + PV are permutation-
        # invariant over keys. This removes the AllGather and makes each
        # core's span independent of multi-core launch skew.
        cm_ps1b = tc.tile_pool(name="ps1b", bufs=1, space="PSUM")
        ps1b = cm_ps1b.__enter__()
        cm_aera = tc.tile_pool(name="aera", bufs=1)
        aera = cm_aera.__enter__()
        cm_wop = tc.tile_pool(name="wop", bufs=1)
        wop = cm_wop.__enter__()

        qT = [
            aera.tile([128, R], bf16, tag=f"qo{mt}", name=f"qT{mt}")
            for mt in range(MT)
        ]
        for mt in range(MT):
            pq = ps1b.tile([128, 512], f32, tag="pkq", bufs=2, name="pq")
            for kt in range(FT):
                nc.tensor.matmul(
                    pq,
                    wq_sb[:, mt, kt * 128 : (kt + 1) * 128],
                    hT[kt],
                    start=(kt == 0),
                    stop=(kt == FT - 1),
                )
            # attention 1/sqrt(D)=0.125 folded into q; attn-LN beta lands
            # via the (pre-scaled) per-partition bias
            nc.scalar.activation(
                out=qT[mt], in_=pq, func=AF.Identity, scale=0.125,
                bias=qb_sb[:, mt : mt + 1],
            )
        cm_wqp.__exit__(None, None, None)

        # foreign-row x: dispatch all 12 loads now (ring waits block only
        # late-needed traffic behind them on each queue)
        cm_xfp = tc.tile_pool(name="xfp", bufs=1, side="right")
        xfp = cm_xfp.__enter__()
        xfs = {}
        for fb in range(RB, KT):
            xf = xfp.tile([128, F], f32, tag="xf", bufs=4, name=f"xf{fb}")
            eng = nc.sync if fb % 2 == 0 else nc.scalar
            eng.dma_start(out=xf, in_=x_d[fb * 128 : (fb + 1) * 128, :])
            xfs[fb] = xf

        # weight prefetch dispatched behind the xf loads: transfers run
        # during attention. wo + w1 first half (scalar q), w1 mid (sync q)
        wo_sb = wop.tile([128, MT, F], bf16, name="wo_sb")
        for mt in range(MT):
            nc.scalar.dma_start(out=wo_sb[:, mt, :], in_=wo_d[mt])
        W1PRE = 16
        W1MID = 8
        w1_sb = w1p.tile([128, W1PRE, FT * 128], bf16, name="w1_sb")
        for mt in range(W1PRE):
            nc.scalar.dma_start(out=w1_sb[:, mt, :], in_=w1_d[mt])
        w1b_sb = w1p.tile([128, W1MID, FT * 128], bf16, name="w1b_sb")

        # kv accumulates into one [128, T] psum region (4 banks); own rows
        # come from hT, foreign rows stream through the LN chain.
        pkva = ps1b.tile([128, T], f32, tag="pkva", bufs=1, name="pkva")
        for rb in range(RB):
            for ft in range(FT):
                nc.tensor.matmul(
                    pkva[:, rb * 128 : (rb + 1) * 128],
                    wkv_sb[:, ft, :],
                    hT[ft][:, rb * 128 : (rb + 1) * 128],
                    start=(ft == 0),
                    stop=(ft == FT - 1),
                )
        # foreign chain is software-pipelined: LN1 of block k+1 issues on
        # the DVE before LN2 of block k (which waits on the gpsimd
        # modulation), so neither engine head-of-line blocks.
        def foreign_head(fb):
            xf = xfs[fb]
            layer_norm(xf, None, None, xf, None)  # in-place normalize
            nc.vector.tensor_mul(out=xf, in0=xf, in1=bc["Wa_bc"])
            nc.gpsimd.tensor_add(out=xf, in0=xf, in1=bc["Ba_bc"])
            return fb

        def foreign_tail(fb):
            xf = xfs[fb]
            hn_bf = work.tile([128, F], bf16, tag="hnbf", bufs=2, name="hnf_bf")
            layer_norm(xf, None, None, hn_bf, None)
            hfT = work.tile([128, F], bf16, tag="hfT", bufs=2, name="hfT")
            for ft in range(FT):
                pt = ps1t.tile([128, 128], bf16, tag="ptt", bufs=2, name="ptt")
                nc.tensor.transpose(
                    pt, hn_bf[:, ft * 128 : (ft + 1) * 128], ident
                )
                nc.scalar.activation(
                    out=hfT[:, ft * 128 : (ft + 1) * 128], in_=pt, func=AF.Copy
                )
            for ft in range(FT):
                nc.tensor.matmul(
                    pkva[:, fb * 128 : (fb + 1) * 128],
                    wkv_sb[:, ft, :],
                    hfT[:, ft * 128 : (ft + 1) * 128],
                    start=(ft == 0),
                    stop=(ft == FT - 1),
                )

        prev_fb = None
        for fb in range(RB, KT):
            foreign_head(fb)
            if prev_fb is not None:
                foreign_tail(prev_fb)
            prev_fb = fb
        foreign_tail(prev_fb)
        cm_xfp.__exit__(None, None, None)

        # ---------------- phase 3: kT / v_ext assembly (SBUF-local) ----------------
        kvTall = work.tile([128, T], bf16, tag="kvTall", bufs=1, name="kvTall")
        for q in range(4):
            nc.scalar.activation(
                out=kvTall[:, q * 512 : (q + 1) * 512],
                in_=pkva[:, q * 512 : (q + 1) * 512],
                func=AF.Identity,
                bias=kvb_sb,
            )
        kT = aera.tile([128, T], bf16, name="kT")
        nc.sync.dma_start(out=kT[0:64, :], in_=kvTall[0:64, :])
        nc.sync.dma_start(out=kT[64:128, :], in_=kvTall[0:64, :])
        vT_sb = work.tile([64, T], bf16, tag="vTs", bufs=1, name="vT_sb")
        nc.sync.dma_start(out=vT_sb, in_=kvTall[64:128, :])
        v_e = [aera.tile([128, 65], bf16, name=f"ve{kt}") for kt in range(KT)]
        v_o = [aera.tile([128, 128], bf16, name=f"vo{kt}") for kt in range(KT)]
        for kt in range(KT):
            nc.vector.memset(v_e[kt][:, 64:65], 1.0)
            nc.vector.memset(v_o[kt], 0.0)
            nc.vector.memset(v_o[kt][:, 32:33], 1.0)
        for kt in range(KT):
            ptv = ps1t.tile([128, 128], bf16, tag="ptt", bufs=2, name="ptv")
            nc.tensor.matmul(
                ptv[:, 0:64],
                vT_sb[:, kt * 128 : (kt + 1) * 128],
                ident[0:64, 0:64],
                is_transpose=True,
            )
            nc.vector.tensor_copy(out=v_e[kt][:, 0:64], in_=ptv[:, 0:64])
            nc.vector.tensor_copy(out=v_o[kt][:, 64:128], in_=ptv[:, 0:64])

        # w1 middle chunk on the sync queue (idle from here to phase 8)
        for mt in range(W1MID):
            nc.sync.dma_start(out=w1b_sb[:, mt, :], in_=w1_d[W1PRE + mt])

        # HAM un-throttle burst: ~5us of dense matmuls reading kT, so it
        # fires exactly when attention becomes runnable and guarantees one
        # fully-busy activity window (the exp-bound attention steady state
        # never reaches 100% PE busy, so it cannot un-throttle itself).
        for i in range(24):
            wps = ps1b.tile([128, 512], f32, tag="pkq", bufs=2, name="warm")
            nc.tensor.matmul(
                wps, kT[0:128, (i % 4) * 512 : (i % 4) * 512 + 128],
                kT[:, (i % 4) * 512 : ((i % 4) + 1) * 512],
                start=True, stop=True,
            )

        cm_ps1b.__exit__(None, None, None)
        cm_ps1t.__exit__(None, None, None)

        # ---------------- phase 4: attention ----------------
        # transposed scores [keys, rows]; heads paired (even at PE rows
        # 0-63, odd at rows 64-127) so MM1 row-tiles 2x. exp covers
        # [128,1024] (two kt) per ACT instruction. Software pipeline:
        # MM1 quad k+1 issues before PV quad k; the softmax tail of pair
        # p is emitted inside pair p+1 so the DVE reciprocal and the bcr
        # broadcast matmuls never stall the PE FIFO.
        cm_ps4 = tc.tile_pool(name="ps4", bufs=1, space="PSUM")
        ps4 = cm_ps4.__enter__()
        cm_attnp = tc.tile_pool(name="attnp", bufs=1)
        attnp = cm_attnp.__enter__()

        outT = [
            aera.tile([128, R], bf16, tag=f"qo{mt}", name=f"outT{mt}")
            for mt in range(MT)
        ]

        def tail_a(st):
            # frees po fast: psum reads first, then the slow reciprocal
            mt, po_e, po_o = st
            t_sb = work.tile([128, R], bf16, tag="tsb", bufs=2, name="t_sb")
            nc.vector.tensor_copy(out=t_sb[0:64, :], in_=po_e[0:64, :])
            nc.vector.tensor_copy(out=t_sb[64:128, :], in_=po_o[64:128, :])
            rcpt = work.tile([128, R], f16, tag="rcpt", bufs=2, name="rcpt")
            with nc.allow_low_precision(reason="f16 softmax reciprocal"):
                nc.vector.reciprocal(out=rcpt[64:65, :], in_=po_e[64:65, :])
                nc.vector.reciprocal(out=rcpt[32:33, :], in_=po_o[32:33, :])
            return mt, t_sb, rcpt

        def tail_b(st2):
            mt, t_sb, rcpt = st2
            bcr = ps4.tile([128, 1024], f32, tag="mm1", bufs=2, name="bcr")
            nc.tensor.matmul(
                bcr[0:64, 0:512], ones2[64:65, :], rcpt[64:65, :],
                start=True, stop=True,
            )
            nc.tensor.matmul(
                bcr[64:128, 0:512], ones2[32:33, :], rcpt[32:33, :],
                start=True, stop=True,
            )
            nc.vector.tensor_mul(
                out=outT[mt][0:64, :], in0=t_sb[0:64, :], in1=bcr[0:64, 0:512]
            )
            nc.vector.tensor_mul(
                out=outT[mt][64:128, :], in0=t_sb[64:128, :], in1=bcr[64:128, 0:512]
            )

        pend = None  # completed pair awaiting tail_a
        pend2 = None  # pair awaiting tail_b
        prev_pv = None  # (kt0, pr_e, pr_o, po_e, po_o) awaiting PV

        def emit_pv(st):
            kt0, pr_e, pr_o, po_e, po_o = st
            for i in range(2):
                kt = kt0 + i
                nc.tensor.matmul(
                    po_e[0:65, :],
                    v_e[kt][:, 0:65],
                    pr_e[:, i * 512 : (i + 1) * 512],
                    start=(kt == 0),
                    stop=(kt == KT - 1),
                )
                nc.tensor.matmul(
                    po_o,
                    v_o[kt],
                    pr_o[:, i * 512 : (i + 1) * 512],
                    start=(kt == 0),
                    stop=(kt == KT - 1),
                )

        for mt in range(MT if STOP >= 4 else 0):
            po_e = ps4.tile([128, 512], f32, tag="po", bufs=3, name="po_e")
            po_o = ps4.tile([128, 512], f32, tag="po", bufs=3, name="po_o")
            for ktt in range(8):
                kt0 = 2 * ktt
                ps_e = ps4.tile([128, 1024], f32, tag="mm1", bufs=2, name="ps_e")
                ps_o = ps4.tile([128, 1024], f32, tag="mm1", bufs=2, name="ps_o")
                for i in range(2):
                    ksl = kT[:, (kt0 + i) * 128 : (kt0 + i + 1) * 128]
                    nc.tensor.matmul(
                        ps_e[:, i * 512 : (i + 1) * 512],
                        ksl[0:64, :],
                        qT[mt][0:64, :],
                        start=True,
                        stop=True,
                    )
                    nc.tensor.matmul(
                        ps_o[:, i * 512 : (i + 1) * 512],
                        ksl[64:128, :],
                        qT[mt][64:128, :],
                        start=True,
                        stop=True,
                    )
                pr_e = attnp.tile([128, 1024], bf16, tag="pr", bufs=3, name="pr_e")
                pr_o = attnp.tile([128, 1024], bf16, tag="pr", bufs=3, name="pr_o")
                nc.scalar.activation(out=pr_e, in_=ps_e, func=AF.Exp)
                nc.scalar.activation(out=pr_o, in_=ps_o, func=AF.Exp)
                if prev_pv is not None:
                    emit_pv(prev_pv)
                prev_pv = (kt0, pr_e, pr_o, po_e, po_o)
                if mt == 0:
                    # hold HAM at 8/8 through the pipeline-fill of the
                    # exp-bound steady state (~74% PE busy can't re-warm)
                    for i in range(2):
                        wps = ps4.tile([128, 512], f32, tag="w4", bufs=1, name="w4")
                        nc.tensor.matmul(
                            wps, kT[0:128, 0:128], kT[:, 0:512],
                            start=True, stop=True,
                        )
                if ktt == 1 and pend is not None:
                    pend2 = tail_a(pend)
                    pend = None
                elif ktt == 3 and pend2 is not None:
                    tail_b(pend2)
                    pend2 = None
            emit_pv(prev_pv)
            prev_pv = None
            pend = (mt, po_e, po_o)
        if pend is not None:
            tail_b(tail_a(pend))
            pend = None

        cm_attnp.__exit__(None, None, None)
        cm_ps4.__exit__(None, None, None)

        # ---------------- phase 5+6: out proj -> x1 -> adaLN-2 ----------------
        cm_x1p = tc.tile_pool(name="x1p", bufs=1, side="right")
        x1p = cm_x1p.__enter__()
        cm_ps56 = tc.tile_pool(name="ps56", bufs=1, space="PSUM")
        ps56 = cm_ps56.__enter__()

        x1 = [x1p.tile([128, F], f32, name=f"x1_{rt}") for rt in range(RB)]
        h2T = [
            hTp.tile([128, R], bf16, tag=f"hT{ft}", name=f"h2T{ft}")
            for ft in range(FT)
        ]
        # even and odd heads accumulate into SEPARATE psum tiles (two
        # concurrent PE row-tiles must not write the same psum addresses);
        # the DVE merges them into x1.
        for rt in range(RB if STOP >= 5 else 0):
            px_e = ps56.tile([128, F], f32, tag="pxe", bufs=2, name="px_e")
            px_o = ps56.tile([128, F], f32, tag="pxo", bufs=1, name="px_o")
            rsl = slice(rt * 128, (rt + 1) * 128)
            for mt in range(MT):
                for nh in range(2):
                    fsl = slice(nh * 512, (nh + 1) * 512)
                    nc.tensor.matmul(
                        px_e[:, fsl],
                        outT[mt][0:64, rsl],
                        wo_sb[0:64, mt, fsl],
                        start=(mt == 0),
                        stop=False,
                    )
                    nc.tensor.matmul(
                        px_o[:, fsl],
                        outT[mt][64:128, rsl],
                        wo_sb[64:128, mt, fsl],
                        start=(mt == 0),
                        stop=(mt == MT - 1),
                    )
            # wo bias via ones-row matmul closes the even accumulation
            for nh in range(2):
                fsl = slice(nh * 512, (nh + 1) * 512)
                nc.tensor.matmul(
                    px_e[:, fsl], onescol, wob_sb[:, fsl],
                    start=False, stop=True,
                )
            nc.vector.tensor_add(out=x1[rt], in0=px_e, in1=h_res[rt])
            nc.vector.tensor_add(out=x1[rt], in0=x1[rt], in1=px_o)
            if STOP < 6:
                continue
            h2_bf = work.tile([128, F], bf16, tag="hnbf", bufs=2, name="h2_bf")
            layer_norm(x1[rt], bc["Wf_bc"], bc["Bf_bc"], h2_bf, nc.gpsimd)
            transpose_to(ps56, 2, h2_bf, h2T, rt)

        cm_ps56.__exit__(None, None, None)
        cm_wop.__exit__(None, None, None)
        cm_aera.__exit__(None, None, None)
        cm_hera.__exit__(None, None, None)

        # ---------------- phase 7: mlp1 + gelu ----------------
        cm_ps78 = tc.tile_pool(name="ps78", bufs=1, space="PSUM")
        ps78 = cm_ps78.__enter__()

        w1tail = {}
        for mt in range(W1PRE + W1MID, MFT if STOP >= 7 else 0):
            t = work.tile([128, FT * 128], bf16, tag="w1c", bufs=4, name="w1c")
            nc.sync.dma_start(out=t, in_=w1_d[mt])
            w1tail[mt] = t

        # w2 even chunks stream on the sync queue (no compute there, so
        # ring-slot waits cannot deadlock); odd chunks dispatch on the
        # scalar queue AFTER the gelus (a dispatch before them would wait
        # on phase-8 matmuls that wait on the gelus -> queue deadlock).
        w2c = {}
        for fh in range(2 if STOP >= 8 else 0):
            for kt in range(0, MFT, 2):
                t = work.tile([128, 512], bf16, tag="w2cs", bufs=3, name="w2cs")
                nc.sync.dma_start(
                    out=t,
                    in_=w2_d[kt * 128 : (kt + 1) * 128, fh * 512 : (fh + 1) * 512],
                )
                w2c[(fh, kt)] = t

        cm_g1p = tc.tile_pool(name="g1p", bufs=1, side="right")
        g1p = cm_g1p.__enter__()
        g1T = [g1p.tile([128, R], bf16, name=f"g1T{mt}") for mt in range(MFT)]
        for mt in range(MFT if STOP >= 7 else 0):
            wsrc = (
                w1_sb[:, mt, :] if mt < W1PRE
                else w1b_sb[:, mt - W1PRE, :] if mt < W1PRE + W1MID
                else w1tail[mt]
            )
            pg = ps78.tile([128, 512], f32, tag="pg", bufs=4, name="pg")
            for kt in range(FT):
                nc.tensor.matmul(
                    pg,
                    wsrc[:, kt * 128 : (kt + 1) * 128],
                    h2T[kt],
                    start=(kt == 0),
                    stop=(kt == FT - 1),
                )
            if os.environ.get("SIM_SAFE"):
                nc.scalar.activation(out=g1T[mt], in_=pg, func=AF.Exp)
            else:
                nc.scalar.activation(
                    out=g1T[mt], in_=pg, func=AF.Gelu,
                    bias=b1_sb[:, mt : mt + 1], scale=1.0,
                )

        cm_w1p.__exit__(None, None, None)
        cm_hTp.__exit__(None, None, None)

        # ---------------- phase 8: mlp2 + residual -> y ----------------
        for fh in range(2 if STOP >= 8 else 0):
            for kt in range(1, MFT, 2):
                t = work.tile([128, 512], bf16, tag="w2ca", bufs=3, name="w2ca")
                nc.scalar.dma_start(
                    out=t,
                    in_=w2_d[kt * 128 : (kt + 1) * 128, fh * 512 : (fh + 1) * 512],
                )
                w2c[(fh, kt)] = t

        if STOP < 8:
            for rt in range(RB):
                yh = work.tile([128, F], f32, tag="ydummy", bufs=2, name="ydummy")
                nc.vector.memset(yh, 0.0)
                nc.sync.dma_start(out=y_d[rt * 128 : (rt + 1) * 128, :], in_=yh)
        for fh in range(2 if STOP >= 8 else 0):
            pf = {}
            for rt in range(RB):
                pf[rt] = ps78.tile([128, 512], f32, tag="pg", bufs=4, name=f"pf{rt}")
            for kt in range(MFT):
                for rt in range(RB):
                    nc.tensor.matmul(
                        pf[rt],
                        g1T[kt][:, rt * 128 : (rt + 1) * 128],
                        w2c[(fh, kt)],
                        start=(kt == 0),
                        stop=False,
                    )
            fsl = slice(fh * 512, (fh + 1) * 512)
            for rt in range(RB):
                nc.tensor.matmul(
                    pf[rt], onescol, b2_sb[:, fsl], start=False, stop=True
                )
            for rt in range(RB):
                yh = work.tile([128, 512], f32, tag="yh", bufs=2, name="yh")
                nc.vector.tensor_add(out=yh, in0=pf[rt], in1=x1[rt][:, fsl])
                nc.sync.dma_start(out=y_d[rt * 128 : (rt + 1) * 128, fsl], in_=yh)

        cm_g1p.__exit__(None, None, None)
        cm_x1p.__exit__(None, None, None)
        cm_ps78.__exit__(None, None, None)

    nc.compile()
    return nc


def _prep_in_maps(inputs):
    f32 = np.float32
    wmod = np.concatenate(
        [inputs["amod_gw"], inputs["amod_bw"], inputs["fmod_gw"], inputs["fmod_bw"]],
        axis=1,
    ).astype(BF16)
    modb = np.concatenate(
        [inputs["amod_gb"], inputs["amod_bb"], inputs["fmod_gb"], inputs["fmod_bb"]]
    ).astype(BF16)
    lnvec = np.stack(
        [
            inputs["amod_nw"],
            inputs["amod_nb"],
            inputs["attn_nw"],
            inputs["attn_nb"],
            inputs["fmod_nw"],
            inputs["fmod_nb"],
        ]
    ).astype(f32)
    # fold the attention-internal LN gamma/beta into wq/wkv
    anw = np.asarray(inputs["attn_nw"]).astype(f32)
    anb = np.asarray(inputs["attn_nb"]).astype(f32)
    wq_f = np.asarray(inputs["wq"]).astype(f32)
    wkv_f = np.asarray(inputs["wkv"]).astype(f32)
    wq_eff = (wq_f * anw[:, None]).astype(BF16)
    wkv_eff = (wkv_f * anw[:, None]).astype(BF16)
    qbias = (anb @ wq_f).astype(f32) * 0.125  # qT copy applies scale=0.125
    kvbias = (anb @ wkv_f).astype(f32).reshape(2 * D, 1)
    wq_t = np.ascontiguousarray(
        wq_eff.reshape(FT, 128, MT, 128)
        .transpose(2, 1, 0, 3).reshape(MT, 128, FT * 128)
    )
    w1_t = np.ascontiguousarray(
        np.asarray(inputs["w1"]).astype(BF16).reshape(FT, 128, MFT, 128)
        .transpose(2, 1, 0, 3).reshape(MFT, 128, FT * 128)
    )
    # wo [H*D, F] -> pair layout [MT, 128, F]: partitions 0-63 = head 2i,
    # 64-127 = head 2i+1.
    wo = np.asarray(inputs["wo"]).astype(BF16).reshape(H, D, F)
    wo_t = np.ascontiguousarray(
        np.stack([np.concatenate([wo[2 * i], wo[2 * i + 1]], 0) for i in range(MT)])
    )
    shared = dict(
        wmod=wmod,
        modb=modb,
        lnvec16=lnvec.astype(BF16),
        wq=wq_t,
        qbias=qbias,
        wkv=wkv_eff,
        kvbias=kvbias,
        wo=wo_t,
        wo_bias=np.asarray(inputs["wo_b"]).astype(BF16).reshape(1, F),
        w1=w1_t,
        b1=np.asarray(inputs["b1"]).astype(f32),
        w2=np.asarray(inputs["w2"]).astype(BF16),
        b2=np.asarray(inputs["b2"]).astype(BF16).reshape(1, F),
    )
    x = np.asarray(inputs["x"]).astype(f32)
    cond = np.asarray(inputs["cond"]).astype(BF16)
    in_maps = []
    for c in range(NCORES):
        b, r0 = c // 4, (c % 4) * R
        m = dict(shared)
        xb = x[b]
        # rotated full batch: own shard first; key order is per-core but
        # attention is permutation-invariant over keys
        m["x"] = np.ascontiguousarray(np.concatenate([xb[r0:], xb[:r0]], 0))
        m["cond"] = np.ascontiguousarray(cond[b])
        in_maps.append(m)
    return in_maps


def _run(inputs, trace=False):
    from concourse.bass_utils import run_bass_kernel_spmd

    if "nc" not in _CACHE:
        _CACHE["nc"] = _build_nc()
    nc = _CACHE["nc"]
    in_maps = _prep_in_maps(inputs)
    res = run_bass_kernel_spmd(
        nc, in_maps, core_ids=list(range(NCORES)), trace=trace
    )
    y = np.empty((B, T, F), np.float32)
    for c in range(NCORES):
        b, r0 = c // 4, (c % 4) * R
        y[b, r0 : r0 + R, :] = res.results[c]["y"]
    return y, res


def kernel(**inputs) -> np.ndarray:
    y, _ = _run(inputs, trace=False)
    return y


if __name__ == "__main__":
    _build_nc()
    print("build OK")


# revision 42
# speedup vs baseline: 1.0123x; 1.0123x over previous
"""DiT block kernel for 8x Trainium2 NeuronCores (Bass/Tile).

Sharding: row-parallel over the flattened (B,T)=4096 rows; 512 rows/core.
Cores 0-3 handle batch 0, cores 4-7 batch 1. MQA K/V is computed per-shard
and AllGather'd within each 4-core batch group. Weights are replicated and
cast to bf16; LN/residual math stays fp32.

v3 notes (on top of v2's row-tiled MM1/outproj, 2-bank exp, prefetch):
  - attention is software-pipelined: MM1 of quad k+1 issues before PV of
    quad k so the scalar-engine exp stream never starves; the per-pair
    softmax tail (denominator copies -> one batched reciprocal -> bcr
    broadcast matmul -> output muls) is deferred into the next pair so
    the slow DVE reciprocal never stalls the PE FIFO.
  - attn-LN's gamma/beta are folded into wq/wkv host-side (bias lands
    via the per-partition bias operand of the qT/kvT PSUM-copy), so the
    second LN is just normalize (saves 2 DVE passes/row-block + 8KB).
  - kvT bounce rides the gpsimd queue right before the AllGather, so the
    collective no longer waits behind megabytes of weight DMA.
  - DMA queues balanced so x/wmod/wq all land before their consumers.
"""

import os
import sys

sys.path.insert(0, "/opt/trn_rl_repo")

import numpy as np
import ml_dtypes

BF16 = ml_dtypes.bfloat16

B, T, F, H, D, M, C = 2, 2048, 1024, 16, 64, 4, 512
NCORES = 8
R = (B * T) // NCORES  # 512 rows per core
RB = R // 128  # 4 row blocks
FT = F // 128  # 8 feature tiles
MT = (H * D) // 128  # 8 head-pair tiles
MFT = (M * F) // 128  # 32 mlp hidden tiles
KT = T // 128  # 16 key tiles
EPS = 1e-5

_CACHE = {}


def _build_nc():
    import concourse.bass as bass
    import concourse.tile as tile
    from concourse import bacc, mybir
    from concourse.masks import make_identity
    from contextlib import ExitStack

    f32 = mybir.dt.float32
    f16 = mybir.dt.float16
    bf16 = mybir.dt.bfloat16
    AF = mybir.ActivationFunctionType
    OP = mybir.AluOpType

    STOP = int(os.environ.get("STOP_AFTER", "99"))

    nc = bacc.Bacc(
        "TRN2",
        target_bir_lowering=False,
        debug=False,
        enable_asserts=False,
        num_devices=NCORES,
    )

    def dram(name, shape, dt, **kw):
        return nc.dram_tensor(name, shape, dt, **kw).ap()

    x_d = dram("x", [T, F], f32, kind="ExternalInput")
    cond_d = dram("cond", [C], bf16, kind="ExternalInput")
    wmod_d = dram("wmod", [C, 4 * F], bf16, kind="ExternalInput")
    modb_d = dram("modb", [4 * F], bf16, kind="ExternalInput")
    lnv16_d = dram("lnvec16", [6, F], bf16, kind="ExternalInput")
    wq_d = dram("wq", [MT, 128, FT * 128], bf16, kind="ExternalInput")
    qb_d = dram("qbias", [H * D], f32, kind="ExternalInput")
    wkv_d = dram("wkv", [F, 2 * D], bf16, kind="ExternalInput")
    kvb_d = dram("kvbias", [2 * D, 1], f32, kind="ExternalInput")
    # wo pre-paired: [pair, 128(d of even head | d of odd head), F]
    wo_d = dram("wo", [MT, 128, F], bf16, kind="ExternalInput")
    wob_d = dram("wo_bias", [1, F], bf16, kind="ExternalInput")
    w1_d = dram("w1", [MFT, 128, FT * 128], bf16, kind="ExternalInput")
    b1_d = dram("b1", [M * F], f32, kind="ExternalInput")
    w2_d = dram("w2", [M * F, F], bf16, kind="ExternalInput")
    b2_d = dram("b2", [1, F], bf16, kind="ExternalInput")
    y_d = dram("y", [R, F], f32, kind="ExternalOutput")

    groups = [[0, 1, 2, 3], [4, 5, 6, 7]]

    def bcast_row(ap_row):
        # [1, n] DRAM AP -> partition-broadcast [128, n]
        return bass.AP(
            tensor=ap_row.tensor,
            offset=ap_row.offset,
            ap=[[0, 128]] + list(ap_row.ap[-1:]),
        )

    with tile.TileContext(nc) as tc, ExitStack() as ctx:
        # left stack: consts, work, hTp, w1p, hera, aera, wop, [attnp]
        # right stack: wqp, xp, [modtmp], then x1p, g1p
        consts = ctx.enter_context(tc.tile_pool(name="consts", bufs=1))
        work = ctx.enter_context(tc.tile_pool(name="work", bufs=2))
        cm_hTp = tc.tile_pool(name="hTp", bufs=1)
        hTp = cm_hTp.__enter__()
        cm_w1p = tc.tile_pool(name="w1p", bufs=1)
        w1p = cm_w1p.__enter__()
        cm_hera = tc.tile_pool(name="hera", bufs=1)
        hera = cm_hera.__enter__()
        cm_wqp = tc.tile_pool(name="wqp", bufs=1, side="right")
        wqp = cm_wqp.__enter__()
        cm_xp = tc.tile_pool(name="xp", bufs=1, side="right")
        xp = cm_xp.__enter__()
        cm_modtmp = tc.tile_pool(name="modtmp", bufs=1, side="right")
        modtmp = cm_modtmp.__enter__()
        dramp = ctx.enter_context(tc.tile_pool(name="dramp", bufs=1, space="DRAM"))

        # ---------------- constants ----------------
        ident = consts.tile([128, 128], bf16, name="ident")
        make_identity(nc, ident)
        ones16 = consts.tile([1, 128], f16, name="ones16")
        nc.vector.memset(ones16, 1.0)
        onescol = consts.tile([1, 128], bf16, name="onescol")
        nc.vector.memset(onescol, 1.0)
        epst = consts.tile([128, 1], f32, name="epst")
        nc.vector.memset(epst, EPS)
        # ones rows at partitions 64 (even-head denom) and 32 (odd-head denom)
        ones2 = consts.tile([128, 64], f16, name="ones2")
        nc.vector.memset(ones2[64:65, :], 1.0)
        nc.vector.memset(ones2[32:33, :], 1.0)

        # cond + wmod grp0 first on both queues (the Wa/Ba modulation
        # gate phase 1's DVE chain), x0 next, then the rest.
        cond_sb = consts.tile([128, 4], bf16, name="cond_sb")
        nc.sync.dma_start(out=cond_sb, in_=cond_d.rearrange("(a p) -> p a", p=128))
        # wmod chunks: grp0 split across sync (ch0/ch2, interleaved with x)
        # and scalar (ch1/ch3); grp1 on scalar. x row blocks on sync.
        def wm_dma(eng, grp, ch):
            wm = modtmp.tile([128, 2048], bf16, tag="wm", bufs=4, name=f"wm{grp}{ch}")
            eng.dma_start(
                out=wm,
                in_=wmod_d[ch * 128 : (ch + 1) * 128, grp * 2048 : (grp + 1) * 2048],
            )
            return wm

        xs = []

        def x_dma(rb):
            x_rb = xp.tile([128, F], f32, name=f"x{rb}")
            nc.sync.dma_start(out=x_rb, in_=x_d[rb * 128 : (rb + 1) * 128, :])
            xs.append(x_rb)

        wm0 = {}
        wm0[1] = wm_dma(nc.scalar, 0, 1)
        wm0[3] = wm_dma(nc.scalar, 0, 3)
        wm0[0] = wm_dma(nc.sync, 0, 0)
        x_dma(0)
        wm0[2] = wm_dma(nc.sync, 0, 2)
        for rb in (1, 2, 3):
            x_dma(rb)
        wm1 = {ch: wm_dma(nc.scalar, 1, ch) for ch in range(4)}
        b1_sb = consts.tile([128, MFT], f32, name="b1_sb")
        nc.sync.dma_start(out=b1_sb, in_=b1_d.rearrange("(mt p) -> p mt", p=128))
        wob_sb = consts.tile([1, F], bf16, name="wob_sb")
        nc.sync.dma_start(out=wob_sb, in_=wob_d)
        b2_sb = consts.tile([1, F], bf16, name="b2_sb")
        nc.sync.dma_start(out=b2_sb, in_=b2_d)
        qb_sb = consts.tile([128, MT], f32, name="qb_sb")
        nc.sync.dma_start(out=qb_sb, in_=qb_d.rearrange("(mt p) -> p mt", p=128))
        kvb_sb = consts.tile([128, 1], f32, name="kvb_sb")
        nc.sync.dma_start(out=kvb_sb, in_=kvb_d)
        lnr = {}
        for r in (0, 1, 4, 5):  # amod_nw/nb, fmod_nw/nb rows at partition 0
            lnr[r] = modtmp.tile([1, F], bf16, name=f"lnr{r}")
            nc.sync.dma_start(out=lnr[r], in_=lnv16_d[r : r + 1, :])
        modb_sb = modtmp.tile([1, 4 * F], bf16, name="modb_sb")
        nc.sync.dma_start(out=modb_sb, in_=modb_d.rearrange("(a f) -> a f", a=1))
        wkv_sb = consts.tile([128, FT, 2 * D], bf16, name="wkv_sb")
        nc.sync.dma_start(
            out=wkv_sb, in_=wkv_d.rearrange("(kt p) n -> p kt n", p=128)
        )



        # wq split across both queues; lands ~45us, q-proj starts ~50us
        wq_sb = wqp.tile([128, MT, FT * 128], bf16, name="wq_sb")
        for mt in range(4):
            nc.sync.dma_start(out=wq_sb[:, mt, :], in_=wq_d[mt])
        for mt in range(4, MT):
            nc.scalar.dma_start(out=wq_sb[:, mt, :], in_=wq_d[mt])

        # ---------------- phase 0: modulation vectors ----------------
        modv = modtmp.tile([1, 4 * F], f16, name="modv")
        tmpv = modtmp.tile([1, F], f16, name="tmpv")
        bc = {}

        def filler(pool, n):
            # dummy matmuls: keep the PE HAM activity window busy
            for _ in range(n):
                wps = pool.tile([128, 512], f32, tag="pmod", bufs=2, name="warm")
                nc.tensor.matmul(wps[:, 0:128], ident, ident, start=True, stop=True)

        def mod_matmuls(ps_pool, grp, wms):
            for jp in range(2):
                pms = [
                    ps_pool.tile([128, 512], f32, tag="pmod", bufs=2, name=f"pm{j}")
                    for j in range(2)
                ]
                for ch in range(4):
                    for j in range(2):
                        nc.tensor.matmul(
                            pms[j][0:1, :],
                            cond_sb[:, ch : ch + 1],
                            wms[ch][:, (2 * jp + j) * 512 : (2 * jp + j + 1) * 512],
                            start=(ch == 0),
                            stop=(ch == 3),
                        )
                with nc.allow_low_precision(reason="f16 modulation vector"):
                    for j in range(2):
                        nb = grp * 4 + 2 * jp + j
                        nc.vector.tensor_add(
                            out=modv[:, nb * 512 : (nb + 1) * 512],
                            in0=pms[j][0:1, :],
                            in1=modb_sb[:, nb * 512 : (nb + 1) * 512],
                        )

        def finalize_mod(ps_pool, g_off, b_off, nw_row, nb_row, w_name, b_name):
            g_sl = modv[:, g_off : g_off + F]
            b_sl = modv[:, b_off : b_off + F]
            with nc.allow_low_precision(reason="f16 modulation vector"):
                nc.scalar.add(out=g_sl, in_=g_sl, add=1.0)
            with nc.allow_low_precision(reason="f16 staging for PE broadcast"):
                nc.vector.tensor_mul(out=tmpv, in0=g_sl, in1=lnr[nb_row])
                nc.vector.tensor_add(out=b_sl, in0=tmpv, in1=b_sl)
                nc.vector.tensor_mul(out=g_sl, in0=g_sl, in1=lnr[nw_row])
            for off, nm in ((g_off, w_name), (b_off, b_name)):
                # Wa/Ba feed f32 DVE/gpsimd chains: mixed-dtype tensor ops
                # run ~3x slower, so keep those two in f32
                bdt = f32 if nm in ("Wa_bc", "Ba_bc") else bf16
                bt = consts.tile([128, F], bdt, name=nm)
                for hf in range(2):
                    pb = ps_pool.tile([128, 512], f32, tag="pmod", bufs=2, name="pbc")
                    nc.tensor.matmul(
                        pb,
                        ones16,
                        modv[:, off + hf * 512 : off + (hf + 1) * 512],
                        start=True,
                        stop=True,
                    )
                    nc.scalar.activation(
                        bt[:, hf * 512 : (hf + 1) * 512], pb, AF.Copy
                    )
                bc[nm] = bt

        cm_ps1t = tc.tile_pool(name="ps1t", bufs=1, space="PSUM")
        ps1t = cm_ps1t.__enter__()
        cm_ps1a = tc.tile_pool(name="ps1a", bufs=1, space="PSUM")
        ps1a = cm_ps1a.__enter__()

        filler(ps1a, 40)
        mod_matmuls(ps1a, 0, wm0)
        finalize_mod(ps1a, 0, F, 0, 1, "Wa_bc", "Ba_bc")

        # ---------------- helpers ----------------
        def layer_norm(src, w_bc, b_bc, out_tile, badd_engine):
            """out = LN(src) * w_bc + b_bc ; src [128,F] f32.
            If w_bc is None: out = plain LN(src) (bf16 ok)."""
            stats = work.tile([128, 2, 6], f32, tag="stats", name="stats")
            for sg in range(2):
                nc.vector.bn_stats(
                    out=stats[:, sg, :], in_=src[:, sg * 512 : (sg + 1) * 512]
                )
            mv = work.tile([128, 2], f32, tag="mv", name="mv")
            nc.vector.bn_aggr(out=mv, in_=stats)
            rstd = work.tile([128, 1], f32, tag="rstd", name="rstd")
            nc.scalar.activation(
                out=rstd, in_=mv[:, 1:2], func=AF.Sqrt, bias=epst, scale=1.0
            )
            nc.vector.reciprocal(out=rstd, in_=rstd)
            tgt = out_tile if w_bc is None else work.tile(
                [128, F], f32, tag="xn", bufs=1, name="xn"
            )
            with nc.allow_low_precision(reason="bf16 normalized activations"):
                nc.vector.tensor_scalar(
                    out=tgt,
                    in0=src,
                    scalar1=mv[:, 0:1],
                    scalar2=rstd,
                    op0=OP.subtract,
                    op1=OP.mult,
                )
            if w_bc is None:
                return
            nc.vector.tensor_mul(out=tgt, in0=tgt, in1=w_bc)
            badd_engine.tensor_add(out=out_tile, in0=tgt, in1=b_bc)

        def transpose_to(ps_pool, bufs, hsrc_bf, hT_tiles, rb):
            """hsrc_bf [128,F] bf16 -> hT_tiles[ft][:, rb*128:+128]."""
            for ft in range(FT):
                pt = ps_pool.tile([128, 128], bf16, tag="ptt", bufs=bufs, name="ptt")
                nc.tensor.transpose(
                    pt, hsrc_bf[:, ft * 128 : (ft + 1) * 128], ident
                )
                nc.scalar.activation(
                    out=hT_tiles[ft][:, rb * 128 : (rb + 1) * 128],
                    in_=pt,
                    func=AF.Copy,
                )

        hT = [
            hTp.tile([128, R], bf16, tag=f"hT{ft}", name=f"hT{ft}")
            for ft in range(FT)
        ]

        # ---------------- phase 1: adaLN-1 + attn-LN + transpose ----------------
        h_res = [hera.tile([128, F], f32, name=f"h{rb}") for rb in range(RB)]
        # per-rb chain, stats first within each block: rb0's full chain
        # completes early so the downstream (kv/foreign) DVE work isn't
        # FIFO-blocked behind later x arrivals
        for rb in range(RB):
            layer_norm(xs[rb], None, None, h_res[rb], None)
            nc.vector.tensor_mul(out=h_res[rb], in0=h_res[rb], in1=bc["Wa_bc"])
            nc.gpsimd.tensor_add(out=h_res[rb], in0=h_res[rb], in1=bc["Ba_bc"])
            hn_bf = work.tile([128, F], bf16, tag="hnbf", bufs=2, name="hn_bf")
            layer_norm(h_res[rb], None, None, hn_bf, None)
            transpose_to(ps1t, 2, hn_bf, hT, rb)
            if rb == 1:
                # fmod modulation slotted here: wm grp1 has landed, and the
                # small DVE/PE work hides between the LN chains
                mod_matmuls(ps1a, 1, wm1)
                finalize_mod(ps1a, 2 * F, 3 * F, 4, 5, "Wf_bc", "Bf_bc")
                cm_modtmp.__exit__(None, None, None)

        cm_xp.__exit__(None, None, None)
        cm_ps1a.__exit__(None, None, None)

        # ---------------- phase 2: q projection + local K/V ----------------
        # no collective: x is the per-core ROTATED full batch (own shard
        # first), so every core computes K/V for all 2048 keys itself.
        # Key order differs per core, but softmax + PV are permutation-
        # invariant over keys. This removes the AllGather and makes each
        # core's span independent of multi-core launch skew.
        cm_ps1b = tc.tile_pool(name="ps1b", bufs=1, space="PSUM")
        ps1b = cm_ps1b.__enter__()
        cm_aera = tc.tile_pool(name="aera", bufs=1)
        aera = cm_aera.__enter__()
        cm_wop = tc.tile_pool(name="wop", bufs=1)
        wop = cm_wop.__enter__()

        qT = [
            aera.tile([128, R], bf16, tag=f"qo{mt}", name=f"qT{mt}")
            for mt in range(MT)
        ]
        for mt in range(MT):
            pq = ps1b.tile([128, 512], f32, tag="pkq", bufs=2, name="pq")
            for kt in range(FT):
                nc.tensor.matmul(
                    pq,
                    wq_sb[:, mt, kt * 128 : (kt + 1) * 128],
                    hT[kt],
                    start=(kt == 0),
                    stop=(kt == FT - 1),
                )
            # attention 1/sqrt(D)=0.125 folded into q; attn-LN beta lands
            # via the (pre-scaled) per-partition bias
            nc.scalar.activation(
                out=qT[mt], in_=pq, func=AF.Identity, scale=0.125,
                bias=qb_sb[:, mt : mt + 1],
            )
        cm_wqp.__exit__(None, None, None)

        # foreign-row x: dispatch all 12 loads now (ring waits block only
        # late-needed traffic behind them on each queue)
        cm_xfp = tc.tile_pool(name="xfp", bufs=1, side="right")
        xfp = cm_xfp.__enter__()
        xfs = {}
        for fb in range(RB, KT):
            xf = xfp.tile([128, F], f32, tag="xf", bufs=4, name=f"xf{fb}")
            eng = nc.sync if fb % 2 == 0 else nc.scalar
            eng.dma_start(out=xf, in_=x_d[fb * 128 : (fb + 1) * 128, :])
            xfs[fb] = xf

        # weight prefetch dispatched behind the xf loads: transfers run
        # during attention. wo + w1 first half (scalar q), w1 mid (sync q)
        wo_sb = wop.tile([128, MT, F], bf16, name="wo_sb")
        for mt in range(MT):
            nc.scalar.dma_start(out=wo_sb[:, mt, :], in_=wo_d[mt])
        W1PRE = 16
        W1MID = 8
        w1_sb = w1p.tile([128, W1PRE, FT * 128], bf16, name="w1_sb")
        for mt in range(W1PRE):
            nc.scalar.dma_start(out=w1_sb[:, mt, :], in_=w1_d[mt])
        w1b_sb = w1p.tile([128, W1MID, FT * 128], bf16, name="w1b_sb")

        # kv accumulates into one [128, T] psum region (4 banks); own rows
        # come from hT, foreign rows stream through the LN chain.
        pkva = ps1b.tile([128, T], f32, tag="pkva", bufs=1, name="pkva")
        for rb in range(RB):
            for ft in range(FT):
                nc.tensor.matmul(
                    pkva[:, rb * 128 : (rb + 1) * 128],
                    wkv_sb[:, ft, :],
                    hT[ft][:, rb * 128 : (rb + 1) * 128],
                    start=(ft == 0),
                    stop=(ft == FT - 1),
                )
        # foreign chain is software-pipelined: LN1 of block k+1 issues on
        # the DVE before LN2 of block k (which waits on the gpsimd
        # modulation), so neither engine head-of-line blocks.
        def foreign_head(fb):
            xf = xfs[fb]
            layer_norm(xf, None, None, xf, None)  # in-place normalize
            nc.vector.tensor_mul(out=xf, in0=xf, in1=bc["Wa_bc"])
            nc.gpsimd.tensor_add(out=xf, in0=xf, in1=bc["Ba_bc"])
            return fb

        def foreign_tail(fb):
            xf = xfs[fb]
            hn_bf = work.tile([128, F], bf16, tag="hnbf", bufs=2, name="hnf_bf")
            layer_norm(xf, None, None, hn_bf, None)
            hfT = work.tile([128, F], bf16, tag="hfT", bufs=2, name="hfT")
            for ft in range(FT):
                pt = ps1t.tile([128, 128], bf16, tag="ptt", bufs=2, name="ptt")
                nc.tensor.transpose(
                    pt, hn_bf[:, ft * 128 : (ft + 1) * 128], ident
                )
                nc.scalar.activation(
                    out=hfT[:, ft * 128 : (ft + 1) * 128], in_=pt, func=AF.Copy
                )
            for ft in range(FT):
                nc.tensor.matmul(
                    pkva[:, fb * 128 : (fb + 1) * 128],
                    wkv_sb[:, ft, :],
                    hfT[:, ft * 128 : (ft + 1) * 128],
                    start=(ft == 0),
                    stop=(ft == FT - 1),
                )

        prev_fb = None
        for fb in range(RB, KT):
            foreign_head(fb)
            if prev_fb is not None:
                foreign_tail(prev_fb)
            prev_fb = fb
        foreign_tail(prev_fb)
        cm_xfp.__exit__(None, None, None)

        # ---------------- phase 3: kT / v_ext assembly (SBUF-local) ----------------
        kvTall = work.tile([128, T], bf16, tag="kvTall", bufs=1, name="kvTall")
        for q in range(4):
            nc.scalar.activation(
                out=kvTall[:, q * 512 : (q + 1) * 512],
                in_=pkva[:, q * 512 : (q + 1) * 512],
                func=AF.Identity,
                bias=kvb_sb,
            )
        kT = aera.tile([128, T], bf16, name="kT")
        nc.sync.dma_start(out=kT[0:64, :], in_=kvTall[0:64, :])
        nc.sync.dma_start(out=kT[64:128, :], in_=kvTall[0:64, :])
        vT_sb = work.tile([64, T], bf16, tag="vTs", bufs=1, name="vT_sb")
        nc.sync.dma_start(out=vT_sb, in_=kvTall[64:128, :])
        v_e = [aera.tile([128, 65], bf16, name=f"ve{kt}") for kt in range(KT)]
        v_o = [aera.tile([128, 128], bf16, name=f"vo{kt}") for kt in range(KT)]
        for kt in range(KT):
            nc.vector.memset(v_e[kt][:, 64:65], 1.0)
            nc.vector.memset(v_o[kt], 0.0)
            nc.vector.memset(v_o[kt][:, 32:33], 1.0)
        for kt in range(KT):
            ptv = ps1t.tile([128, 128], bf16, tag="ptt", bufs=2, name="ptv")
            nc.tensor.matmul(
                ptv[:, 0:64],
                vT_sb[:, kt * 128 : (kt + 1) * 128],
                ident[0:64, 0:64],
                is_transpose=True,
            )
            nc.vector.tensor_copy(out=v_e[kt][:, 0:64], in_=ptv[:, 0:64])
            nc.vector.tensor_copy(out=v_o[kt][:, 64:128], in_=ptv[:, 0:64])

        # w1 middle chunk on the sync queue (idle from here to phase 8)
        for mt in range(W1MID):
            nc.sync.dma_start(out=w1b_sb[:, mt, :], in_=w1_d[W1PRE + mt])

        # HAM un-throttle burst: ~5us of dense matmuls reading kT, so it
        # fires exactly when attention becomes runnable and guarantees one
        # fully-busy activity window (the exp-bound attention steady state
        # never reaches 100% PE busy, so it cannot un-throttle itself).
        for i in range(24):
            wps = ps1b.tile([128, 512], f32, tag="pkq", bufs=2, name="warm")
            nc.tensor.matmul(
                wps, kT[0:128, (i % 4) * 512 : (i % 4) * 512 + 128],
                kT[:, (i % 4) * 512 : ((i % 4) + 1) * 512],
                start=True, stop=True,
            )

        cm_ps1b.__exit__(None, None, None)
        cm_ps1t.__exit__(None, None, None)

        # ---------------- phase 4: attention ----------------
        # transposed scores [keys, rows]; heads paired (even at PE rows
        # 0-63, odd at rows 64-127) so MM1 row-tiles 2x. exp covers
        # [128,1024] (two kt) per ACT instruction. Software pipeline:
        # MM1 quad k+1 issues before PV quad k; the softmax tail of pair
        # p is emitted inside pair p+1 so the DVE reciprocal and the bcr
        # broadcast matmuls never stall the PE FIFO.
        cm_ps4 = tc.tile_pool(name="ps4", bufs=1, space="PSUM")
        ps4 = cm_ps4.__enter__()
        cm_attnp = tc.tile_pool(name="attnp", bufs=1)
        attnp = cm_attnp.__enter__()

        outT = [
            aera.tile([128, R], bf16, tag=f"qo{mt}", name=f"outT{mt}")
            for mt in range(MT)
        ]

        def tail_a(st):
            # frees po fast: psum reads first, then the slow reciprocal
            mt, po_e, po_o = st
            t_sb = work.tile([128, R], bf16, tag="tsb", bufs=2, name="t_sb")
            nc.vector.tensor_copy(out=t_sb[0:64, :], in_=po_e[0:64, :])
            nc.vector.tensor_copy(out=t_sb[64:128, :], in_=po_o[64:128, :])
            rcpt = work.tile([128, R], f16, tag="rcpt", bufs=2, name="rcpt")
            with nc.allow_low_precision(reason="f16 softmax reciprocal"):
                nc.vector.reciprocal(out=rcpt[64:65, :], in_=po_e[64:65, :])
                nc.vector.reciprocal(out=rcpt[32:33, :], in_=po_o[32:33, :])
            return mt, t_sb, rcpt

        def tail_b(st2):
            mt, t_sb, rcpt = st2
            bcr = ps4.tile([128, 1024], f32, tag="mm1", bufs=2, name="bcr")
            nc.tensor.matmul(
                bcr[0:64, 0:512], ones2[64:65, :], rcpt[64:65, :],
                start=True, stop=True,
            )
            nc.tensor.matmul(
                bcr[64:128, 0:512], ones2[32:33, :], rcpt[32:33, :],
                start=True, stop=True,
            )
            nc.vector.tensor_mul(
                out=outT[mt][0:64, :], in0=t_sb[0:64, :], in1=bcr[0:64, 0:512]
            )
            nc.vector.tensor_mul(
                out=outT[mt][64:128, :], in0=t_sb[64:128, :], in1=bcr[64:128, 0:512]
            )

        pend = None  # completed pair awaiting tail_a
        pend2 = None  # pair awaiting tail_b
        prev_pv = None  # (kt0, pr_e, pr_o, po_e, po_o) awaiting PV

        def emit_pv(st):
            kt0, pr_e, pr_o, po_e, po_o = st
            for i in range(2):
                kt = kt0 + i
                nc.tensor.matmul(
                    po_e[0:65, :],
                    v_e[kt][:, 0:65],
                    pr_e[:, i * 512 : (i + 1) * 512],
                    start=(kt == 0),
                    stop=(kt == KT - 1),
                )
                nc.tensor.matmul(
                    po_o,
                    v_o[kt],
                    pr_o[:, i * 512 : (i + 1) * 512],
                    start=(kt == 0),
                    stop=(kt == KT - 1),
                )

        for mt in range(MT if STOP >= 4 else 0):
            po_e = ps4.tile([128, 512], f32, tag="po", bufs=3, name="po_e")
            po_o = ps4.tile([128, 512], f32, tag="po", bufs=3, name="po_o")
            for ktt in range(8):
                kt0 = 2 * ktt
                ps_e = ps4.tile([128, 1024], f32, tag="mm1", bufs=2, name="ps_e")
                ps_o = ps4.tile([128, 1024], f32, tag="mm1", bufs=2, name="ps_o")
                for i in range(2):
                    ksl = kT[:, (kt0 + i) * 128 : (kt0 + i + 1) * 128]
                    nc.tensor.matmul(
                        ps_e[:, i * 512 : (i + 1) * 512],
                        ksl[0:64, :],
                        qT[mt][0:64, :],
                        start=True,
                        stop=True,
                    )
                    nc.tensor.matmul(
                        ps_o[:, i * 512 : (i + 1) * 512],
                        ksl[64:128, :],
                        qT[mt][64:128, :],
                        start=True,
                        stop=True,
                    )
                pr_e = attnp.tile([128, 1024], bf16, tag="pr", bufs=3, name="pr_e")
                pr_o = attnp.tile([128, 1024], bf16, tag="pr", bufs=3, name="pr_o")
                nc.scalar.activation(out=pr_e, in_=ps_e, func=AF.Exp)
                nc.scalar.activation(out=pr_o, in_=ps_o, func=AF.Exp)
                if prev_pv is not None:
                    emit_pv(prev_pv)
                prev_pv = (kt0, pr_e, pr_o, po_e, po_o)
                if mt == 0:
                    # hold HAM at 8/8 through the pipeline-fill of the
                    # exp-bound steady state (~74% PE busy can't re-warm)
                    for i in range(2):
                        wps = ps4.tile([128, 512], f32, tag="w4", bufs=1, name="w4")
                        nc.tensor.matmul(
                            wps, kT[0:128, 0:128], kT[:, 0:512],
                            start=True, stop=True,
                        )
                if ktt == 1 and pend is not None:
                    pend2 = tail_a(pend)
                    pend = None
                elif ktt == 3 and pend2 is not None:
                    tail_b(pend2)
                    pend2 = None
            emit_pv(prev_pv)
            prev_pv = None
            pend = (mt, po_e, po_o)
        if pend is not None:
            tail_b(tail_a(pend))
            pend = None

        cm_attnp.__exit__(None, None, None)
        cm_ps4.__exit__(None, None, None)

        # ---------------- phase 5+6: out proj -> x1 -> adaLN-2 ----------------
        cm_x1p = tc.tile_pool(name="x1p", bufs=1, side="right")
        x1p = cm_x1p.__enter__()
        cm_ps56 = tc.tile_pool(name="ps56", bufs=1, space="PSUM")
        ps56 = cm_ps56.__enter__()

        x1 = [x1p.tile([128, F], f32, name=f"x1_{rt}") for rt in range(RB)]
        h2T = [
            hTp.tile([128, R], bf16, tag=f"hT{ft}", name=f"h2T{ft}")
            for ft in range(FT)
        ]
        # even and odd heads accumulate into SEPARATE psum tiles (two
        # concurrent PE row-tiles must not write the same psum addresses);
        # the DVE merges them into x1.
        for rt in range(RB if STOP >= 5 else 0):
            px_e = ps56.tile([128, F], f32, tag="pxe", bufs=2, name="px_e")
            px_o = ps56.tile([128, F], f32, tag="pxo", bufs=1, name="px_o")
            rsl = slice(rt * 128, (rt + 1) * 128)
            for mt in range(MT):
                for nh in range(2):
                    fsl = slice(nh * 512, (nh + 1) * 512)
                    nc.tensor.matmul(
                        px_e[:, fsl],
                        outT[mt][0:64, rsl],
                        wo_sb[0:64, mt, fsl],
                        start=(mt == 0),
                        stop=False,
                    )
                    nc.tensor.matmul(
                        px_o[:, fsl],
                        outT[mt][64:128, rsl],
                        wo_sb[64:128, mt, fsl],
                        start=(mt == 0),
                        stop=(mt == MT - 1),
                    )
            # wo bias via ones-row matmul closes the even accumulation
            for nh in range(2):
                fsl = slice(nh * 512, (nh + 1) * 512)
                nc.tensor.matmul(
                    px_e[:, fsl], onescol, wob_sb[:, fsl],
                    start=False, stop=True,
                )
            nc.vector.tensor_add(out=x1[rt], in0=px_e, in1=h_res[rt])
            nc.vector.tensor_add(out=x1[rt], in0=x1[rt], in1=px_o)
            if STOP < 6:
                continue
            h2_bf = work.tile([128, F], bf16, tag="hnbf", bufs=2, name="h2_bf")
            layer_norm(x1[rt], bc["Wf_bc"], bc["Bf_bc"], h2_bf, nc.gpsimd)
            transpose_to(ps56, 2, h2_bf, h2T, rt)

        cm_ps56.__exit__(None, None, None)
        cm_wop.__exit__(None, None, None)
        cm_aera.__exit__(None, None, None)
        cm_hera.__exit__(None, None, None)

        # ---------------- phase 7: mlp1 + gelu ----------------
        cm_ps78 = tc.tile_pool(name="ps78", bufs=1, space="PSUM")
        ps78 = cm_ps78.__enter__()

        w1tail = {}
        for mt in range(W1PRE + W1MID, MFT if STOP >= 7 else 0):
            t = work.tile([128, FT * 128], bf16, tag="w1c", bufs=4, name="w1c")
            nc.sync.dma_start(out=t, in_=w1_d[mt])
            w1tail[mt] = t

        # w2 even chunks stream on the sync queue (no compute there, so
        # ring-slot waits cannot deadlock); odd chunks dispatch on the
        # scalar queue AFTER the gelus (a dispatch before them would wait
        # on phase-8 matmuls that wait on the gelus -> queue deadlock).
        w2c = {}
        for fh in range(2 if STOP >= 8 else 0):
            for kt in range(0, MFT, 2):
                t = work.tile([128, 512], bf16, tag="w2cs", bufs=3, name="w2cs")
                nc.sync.dma_start(
                    out=t,
                    in_=w2_d[kt * 128 : (kt + 1) * 128, fh * 512 : (fh + 1) * 512],
                )
                w2c[(fh, kt)] = t

        cm_g1p = tc.tile_pool(name="g1p", bufs=1, side="right")
        g1p = cm_g1p.__enter__()
        g1T = [g1p.tile([128, R], bf16, name=f"g1T{mt}") for mt in range(MFT)]
        for mt in range(MFT if STOP >= 7 else 0):
            wsrc = (
                w1_sb[:, mt, :] if mt < W1PRE
                else w1b_sb[:, mt - W1PRE, :] if mt < W1PRE + W1MID
                else w1tail[mt]
            )
            pg = ps78.tile([128, 512], f32, tag="pg", bufs=4, name="pg")
            for kt in range(FT):
                nc.tensor.matmul(
                    pg,
                    wsrc[:, kt * 128 : (kt + 1) * 128],
                    h2T[kt],
                    start=(kt == 0),
                    stop=(kt == FT - 1),
                )
            if os.environ.get("SIM_SAFE"):
                nc.scalar.activation(out=g1T[mt], in_=pg, func=AF.Exp)
            else:
                nc.scalar.activation(
                    out=g1T[mt], in_=pg, func=AF.Gelu,
                    bias=b1_sb[:, mt : mt + 1], scale=1.0,
                )

        cm_w1p.__exit__(None, None, None)
        cm_hTp.__exit__(None, None, None)

        # ---------------- phase 8: mlp2 + residual -> y ----------------
        for fh in range(2 if STOP >= 8 else 0):
            for kt in range(1, MFT, 2):
                t = work.tile([128, 512], bf16, tag="w2ca", bufs=3, name="w2ca")
                nc.scalar.dma_start(
                    out=t,
                    in_=w2_d[kt * 128 : (kt + 1) * 128, fh * 512 : (fh + 1) * 512],
                )
                w2c[(fh, kt)] = t

        if STOP < 8:
            for rt in range(RB):
                yh = work.tile([128, F], f32, tag="ydummy", bufs=2, name="ydummy")
                nc.vector.memset(yh, 0.0)
                nc.sync.dma_start(out=y_d[rt * 128 : (rt + 1) * 128, :], in_=yh)
        for fh in range(2 if STOP >= 8 else 0):
            pf = {}
            for rt in range(RB):
                pf[rt] = ps78.tile([128, 512], f32, tag="pg", bufs=4, name=f"pf{rt}")
            for kt in range(MFT):
                for rt in range(RB):
                    nc.tensor.matmul(
                        pf[rt],
                        g1T[kt][:, rt * 128 : (rt + 1) * 128],
                        w2c[(fh, kt)],
                        start=(kt == 0),
                        stop=False,
                    )
            fsl = slice(fh * 512, (fh + 1) * 512)
            for rt in range(RB):
                nc.tensor.matmul(
                    pf[rt], onescol, b2_sb[:, fsl], start=False, stop=True
                )
            for rt in range(RB):
                yh = work.tile([128, 512], f32, tag="yh", bufs=2, name="yh")
                nc.vector.tensor_add(out=yh, in0=pf[rt], in1=x1[rt][:, fsl])
                nc.sync.dma_start(out=y_d[rt * 128 : (rt + 1) * 128, fsl], in_=yh)

        cm_g1p.__exit__(None, None, None)
        cm_x1p.__exit__(None, None, None)
        cm_ps78.__exit__(None, None, None)

    nc.compile()
    return nc


def _prep_in_maps(inputs):
    f32 = np.float32
    wmod = np.concatenate(
        [inputs["amod_gw"], inputs["amod_bw"], inputs["fmod_gw"], inputs["fmod_bw"]],
        axis=1,
    ).astype(BF16)
    modb = np.concatenate(
        [inputs["amod_gb"], inputs["amod_bb"], inputs["fmod_gb"], inputs["fmod_bb"]]
    ).astype(BF16)
    lnvec = np.stack(
        [
            inputs["amod_nw"],
            inputs["amod_nb"],
            inputs["attn_nw"],
            inputs["attn_nb"],
            inputs["fmod_nw"],
            inputs["fmod_nb"],
        ]
    ).astype(f32)
    # fold the attention-internal LN gamma/beta into wq/wkv
    anw = np.asarray(inputs["attn_nw"]).astype(f32)
    anb = np.asarray(inputs["attn_nb"]).astype(f32)
    wq_f = np.asarray(inputs["wq"]).astype(f32)
    wkv_f = np.asarray(inputs["wkv"]).astype(f32)
    wq_eff = (wq_f * anw[:, None]).astype(BF16)
    wkv_eff = (wkv_f * anw[:, None]).astype(BF16)
    qbias = (anb @ wq_f).astype(f32) * 0.125  # qT copy applies scale=0.125
    kvbias = (anb @ wkv_f).astype(f32).reshape(2 * D, 1)
    wq_t = np.ascontiguousarray(
        wq_eff.reshape(FT, 128, MT, 128)
        .transpose(2, 1, 0, 3).reshape(MT, 128, FT * 128)
    )
    w1_t = np.ascontiguousarray(
        np.asarray(inputs["w1"]).astype(BF16).reshape(FT, 128, MFT, 128)
        .transpose(2, 1, 0, 3).reshape(MFT, 128, FT * 128)
    )
    # wo [H*D, F] -> pair layout [MT, 128, F]: partitions 0-63 = head 2i,
    # 64-127 = head 2i+1.
    wo = np.asarray(inputs["wo"]).astype(BF16).reshape(H, D, F)
    wo_t = np.ascontiguousarray(
        np.stack([np.concatenate([wo[2 * i], wo[2 * i + 1]], 0) for i in range(MT)])
    )
    shared = dict(
        wmod=wmod,
        modb=modb,
        lnvec16=lnvec.astype(BF16),
        wq=wq_t,
        qbias=qbias,
        wkv=wkv_eff,
        kvbias=kvbias,
        wo=wo_t,
        wo_bias=np.asarray(inputs["wo_b"]).astype(BF16).reshape(1, F),
        w1=w1_t,
        b1=np.asarray(inputs["b1"]).astype(f32),
        w2=np.asarray(inputs["w2"]).astype(BF16),
        b2=np.asarray(inputs["b2"]).astype(BF16).reshape(1, F),
    )
    x = np.asarray(inputs["x"]).astype(f32)
    cond = np.asarray(inputs["cond"]).astype(BF16)
    in_maps = []
    for c in range(NCORES):
        b, r0 = c // 4, (c % 4) * R
        m = dict(shared)
        xb = x[b]
        # rotated full batch: own shard first; key order is per-core but
        # attention is permutation-invariant over keys
        m["x"] = np.ascontiguousarray(np.concatenate([xb[r0:], xb[:r0]], 0))
        m["cond"] = np.ascontiguousarray(cond[b])
        in_maps.append(m)
    return in_maps


def _run(inputs, trace=False):
    from concourse.bass_utils import run_bass_kernel_spmd

    if "nc" not in _CACHE:
        _CACHE["nc"] = _build_nc()
    nc = _CACHE["nc"]
    in_maps = _prep_in_maps(inputs)
    res = run_bass_kernel_spmd(
        nc, in_maps, core_ids=list(range(NCORES)), trace=trace
    )
    y = np.empty((B, T, F), np.float32)
    for c in range(NCORES):
        b, r0 = c // 4, (c % 4) * R
        y[b, r0 : r0 + R, :] = res.results[c]["y"]
    return y, res


def kernel(**inputs) -> np.ndarray:
    y, _ = _run(inputs, trace=False)
    return y


if __name__ == "__main__":
    _build_nc()
    print("build OK")
